# revision 2
# baseline (speedup 1.0000x reference)
"""EwaldBlock Trainium2 kernel — 8-core data-parallel over structures.

Strategy (see sharding hint): batch_seg is sorted, so atoms are contiguous
per structure. Each of the 8 cores owns 8 consecutive structures; every
structure is zero-padded to NS atom slots so all cores run one identical
SPMD program (padded atoms have h=0/x=0 and contribute nothing to the
structure factors; their outputs are dropped on the host).

Per core (NA = 8*NS padded atoms):
  - dot   = per-structure x @ k_b^T via small fp32 matmuls (contraction 3)
  - C/S   = cos/sin(dot) on ACT after Cody-Waite range reduction on DVE
            (ACT Sin is only accurate within [-pi, pi])
  - hres  = pre-residual MLP in E-major layout (features on partitions,
            atoms on the free dim) with float32r matmuls
  - sf    = per-structure C^T @ hres (K x E) accumulated in PSUM
  - hupd  = kfilter-weighted reprojection, directly E-major
  - out   = Dense + 3 residual blocks, E-major, then PE-transpose to
            atom-major and DMA out
"""
import math

import numpy as np

N, B, K, E, D = 4096, 64, 128, 256, 8
NUM_HIDDEN = 3
NCORES = 8
SB = B // NCORES   # structures per core
P = 128
EC = E // P        # feature chunks of 128

TWO_PI = float(2 * np.pi)
INV_2PI = float(np.float32(1.0 / TWO_PI))
MAGIC = float(np.float32(1.5 * 2**23))
C1, C2, C3 = 6.28125, 0.0019350051879882812, 3.019916050561733e-07
PI = float(np.pi)

_cache = {}


def _build(NS):
    import concourse.bass as bass
    import concourse.tile as tile
    import concourse.mybir as mybir
    from concourse import bacc
    from concourse.masks import make_identity

    f32 = mybir.dt.float32
    f32r = mybir.dt.float32r
    AF = mybir.ActivationFunctionType
    ALU = mybir.AluOpType

    assert NS <= P
    NA = SB * NS

    # free-dim chunks for dense matmuls: <=512 and PSUM-bank aligned
    ach = []
    a = 0
    while a < NA:
        w = min(512, NA - a)
        ach.append((a, w))
        a += w

    nc = bacc.Bacc("TRN2", target_bir_lowering=False, debug=False)

    hT_d = nc.dram_tensor("hT", [E, NA], f32r, kind="ExternalInput").ap()
    xT_d = nc.dram_tensor("xT", [3, NA], f32, kind="ExternalInput").ap()
    kT_d = nc.dram_tensor("kT", [SB, 3, K], f32, kind="ExternalInput").ap()
    kfil_d = nc.dram_tensor("kfil", [K, E], f32, kind="ExternalInput").ap()
    WpreT_d = nc.dram_tensor("WpreT", [2, E, E], f32r, kind="ExternalInput").ap()
    W0T_d = nc.dram_tensor("W0T", [E, E], f32r, kind="ExternalInput").ap()
    WresT_d = nc.dram_tensor("WresT", [NUM_HIDDEN, 2, E, E], f32r,
                             kind="ExternalInput").ap()
    hu_d = nc.dram_tensor("hu", [NA, E], f32, kind="ExternalOutput").ap()
    dot_d = nc.dram_tensor("dot", [NA, K], f32, kind="ExternalOutput").ap()

    with tile.TileContext(nc) as tc:
        import contextlib
        with contextlib.ExitStack() as ctx:
            cons = ctx.enter_context(tc.tile_pool(name="cons", bufs=1))
            acts = ctx.enter_context(tc.tile_pool(name="acts", bufs=1))
            work = ctx.enter_context(tc.tile_pool(name="work", bufs=3))
            ps = ctx.enter_context(tc.tile_pool(name="ps", bufs=6, space="PSUM"))

            # ---------- loads ----------
            hT = [cons.tile([P, NA], f32r, name=f"hT{i}") for i in range(EC)]
            for i in range(EC):
                nc.sync.dma_start(hT[i][:], hT_d[i * P:(i + 1) * P, :])
            xT = cons.tile([3, NA], f32, name="xT")
            nc.sync.dma_start(xT[:], xT_d)
            kT = cons.tile([3, SB * K], f32, name="kT")
            for j in range(SB):
                nc.sync.dma_start(kT[:, j * K:(j + 1) * K], kT_d[j])
            kfil = cons.tile([K, E], f32, name="kfil")
            nc.sync.dma_start(kfil[:], kfil_d)

            # transposed weights as lhsT tiles (e_in on partitions)
            wsrc = [WpreT_d[0], WpreT_d[1], W0T_d]
            for hh in range(NUM_HIDDEN):
                wsrc += [WresT_d[hh, 0], WresT_d[hh, 1]]
            Wt = []
            for wi, wd in enumerate(wsrc):
                tiles = []
                for i in range(EC):
                    t_ = cons.tile([P, E], f32r, name=f"w{wi}_{i}")
                    nc.sync.dma_start(t_[:], wd[i * P:(i + 1) * P, :])
                    tiles.append(t_)
                Wt.append(tiles)
            W_PRE0, W_PRE1, W_0 = 0, 1, 2

            ident = cons.tile([P, P], f32, name="ident")
            make_identity(nc, ident[:])
            identr = cons.tile([P, P], f32r, name="identr")
            nc.vector.tensor_copy(identr[:], ident[:])

            # ---------- phase A: dot + trig (ACT table: sin) ----------
            C_at = [acts.tile([NS, K], f32r, name=f"C_at{j}") for j in range(SB)]
            S_at = [acts.tile([NS, K], f32r, name=f"S_at{j}") for j in range(SB)]
            CT = acts.tile([K, NA], f32r, name="CT")
            ST = acts.tile([K, NA], f32r, name="ST")

            for j in range(SB):
                psd = ps.tile([NS, K], f32, tag="mm", name="psd")
                nc.tensor.matmul(psd[:], xT[:, NS * j:NS * (j + 1)],
                                 kT[:, j * K:(j + 1) * K],
                                 start=True, stop=True)
                dsb = acts.tile([NS, K], f32, name=f"dot_sb{j}")
                nc.vector.tensor_copy(dsb[:], psd[:])
                nc.sync.dma_start(dot_d[NS * j:NS * (j + 1), :], dsb[:])
                # range reduction: r = dot - 2*pi*round(dot/(2*pi))
                tk = work.tile([NS, K], f32, name="tk")
                nc.vector.tensor_scalar(tk[:], dsb[:], INV_2PI, MAGIC,
                                        ALU.mult, ALU.add)
                nc.vector.tensor_scalar_sub(tk[:], tk[:], MAGIC)
                rr = work.tile([NS, K], f32, name="rr")
                nc.vector.cody_waite_cascade(rr[:], dsb[:], tk[:], C1, C2, C3)
                cc = work.tile([NS, K], f32, name="cc")
                nc.vector.add_range_wrap(cc[:], rr[:], PI / 2, PI, TWO_PI)
                nc.scalar.activation(S_at[j][:], rr[:], AF.Sin)
                nc.scalar.activation(C_at[j][:], cc[:], AF.Sin)
                # K-major copies via PE transpose
                for src, dst in ((C_at[j], CT), (S_at[j], ST)):
                    pst = ps.tile([K, NS], f32r, tag="mm", name="pst")
                    nc.tensor.transpose(pst[:], src[:], identr[:NS, :NS])
                    nc.vector.tensor_copy(dst[:, NS * j:NS * (j + 1)], pst[:])

            # ---------- helper: dense E->E in E-major ----------
            def dense(widx, src, dst, name):
                for o in range(EC):
                    for (a0, aw) in ach:
                        pd = ps.tile([P, 512], f32, tag="mm", name=f"pd_{name}")
                        for i in range(EC):
                            nc.tensor.matmul(
                                pd[:, :aw],
                                Wt[widx][i][:, o * P:(o + 1) * P],
                                src[i][:, a0:a0 + aw],
                                start=(i == 0), stop=(i == EC - 1))
                        nc.scalar.activation(dst[o][:, a0:a0 + aw], pd[:, :aw],
                                             AF.Silu)

            # ---------- phase B1: pre-residual MLP ----------
            y1T = [acts.tile([P, NA], f32r, name=f"y1T{i}") for i in range(EC)]
            dense(W_PRE0, hT, y1T, "pre0")
            y2T = [acts.tile([P, NA], f32r, name=f"y2T{i}") for i in range(EC)]
            dense(W_PRE1, y1T, y2T, "pre1")
            hresT = [acts.tile([P, NA], f32r, name=f"hresT{i}") for i in range(EC)]
            for i in range(EC):
                nc.vector.tensor_add(hresT[i][:], hT[i][:], y2T[i][:])
            # atom-major hres via PE transpose (per structure)
            hres_at = [acts.tile([NS, E], f32r, name=f"hres_at{j}")
                       for j in range(SB)]
            for i in range(EC):
                for j in range(SB):
                    psh = ps.tile([NS, P], f32r, tag="mm", name="psh")
                    nc.tensor.transpose(psh[:], hresT[i][:, NS * j:NS * (j + 1)],
                                        identr[:])
                    nc.scalar.copy(hres_at[j][:, i * P:(i + 1) * P], psh[:])

            # ---------- phase B2: structure factors + reprojection ----------
            huT = [acts.tile([P, NA], f32r, name=f"huT{i}") for i in range(EC)]
            for j in range(SB):
                ps_r = ps.tile([K, E], f32, tag="mm", name="ps_r")
                ps_i = ps.tile([K, E], f32, tag="mm", name="ps_i")
                nc.tensor.matmul(ps_r[:], C_at[j][:], hres_at[j][:],
                                 start=True, stop=True)
                nc.tensor.matmul(ps_i[:], S_at[j][:], hres_at[j][:],
                                 start=True, stop=True)
                F_r = work.tile([K, E], f32r, name="F_r")
                nc.vector.tensor_mul(F_r[:], ps_r[:], kfil[:])
                F_i = work.tile([K, E], f32r, name="F_i")
                nc.vector.tensor_mul(F_i[:], ps_i[:], kfil[:])
                for o in range(EC):
                    ph = ps.tile([P, NS], f32, tag="mm", name="ph")
                    nc.tensor.matmul(ph[:], F_r[:, o * P:(o + 1) * P],
                                     CT[:, j * NS:(j + 1) * NS],
                                     start=True, stop=False)
                    nc.tensor.matmul(ph[:], F_i[:, o * P:(o + 1) * P],
                                     ST[:, j * NS:(j + 1) * NS],
                                     start=False, stop=True)
                    nc.vector.tensor_copy(huT[o][:, j * NS:(j + 1) * NS], ph[:])

            # ---------- phase B3: output MLP ----------
            cur = [acts.tile([P, NA], f32r, name=f"a0T{i}") for i in range(EC)]
            dense(W_0, huT, cur, "w0")
            for hh in range(NUM_HIDDEN):
                ya = [acts.tile([P, NA], f32r, name=f"ya{hh}_{i}") for i in range(EC)]
                dense(3 + 2 * hh, cur, ya, f"r{hh}a")
                yb = [acts.tile([P, NA], f32r, name=f"yb{hh}_{i}") for i in range(EC)]
                dense(4 + 2 * hh, ya, yb, f"r{hh}b")
                nxt = [acts.tile([P, NA], f32r, name=f"xn{hh}_{i}") for i in range(EC)]
                for i in range(EC):
                    nc.vector.tensor_add(nxt[i][:], cur[i][:], yb[i][:])
                cur = nxt

            # transpose to atom-major and store (per structure)
            for j in range(SB):
                hu_at = work.tile([NS, E], f32, name="hu_at")
                for i in range(EC):
                    psf = ps.tile([NS, P], f32r, tag="mm", name="psf")
                    nc.tensor.transpose(psf[:], cur[i][:, NS * j:NS * (j + 1)],
                                        identr[:])
                    nc.scalar.copy(hu_at[:, i * P:(i + 1) * P], psf[:])
                nc.sync.dma_start(hu_d[NS * j:NS * (j + 1), :], hu_at[:])

    nc.compile()
    return nc


def kernel(h, x, k, batch_seg, num_batch, W_pre, W_down, W_up, W0, W_res):
    from concourse.bass_utils import run_bass_kernel_spmd

    h = np.ascontiguousarray(np.asarray(h, dtype=np.float32))
    x = np.ascontiguousarray(np.asarray(x, dtype=np.float32))
    k = np.ascontiguousarray(np.asarray(k, dtype=np.float32))
    batch_seg = np.asarray(batch_seg).astype(np.int64)
    W_pre = np.asarray(W_pre, dtype=np.float32)
    W_down = np.asarray(W_down, dtype=np.float32)
    W_up = np.asarray(W_up, dtype=np.float32)
    W0 = np.asarray(W0, dtype=np.float32)
    W_res = np.asarray(W_res, dtype=np.float32)

    assert np.all(np.diff(batch_seg) >= 0), "batch_seg must be sorted"
    counts = np.bincount(batch_seg, minlength=B)
    starts = np.zeros(B + 1, np.int64)
    starts[1:] = np.cumsum(counts)
    NS = max(96, int(math.ceil(counts.max() / 16)) * 16)
    NA = SB * NS

    kfil = np.ascontiguousarray((W_up @ W_down).T.astype(np.float32))  # (K,E)
    WpreT = np.ascontiguousarray(W_pre.transpose(0, 2, 1))
    W0T = np.ascontiguousarray((0.01 * W0).T)                          # fold 0.01
    WresT = np.ascontiguousarray(W_res.transpose(0, 1, 3, 2))

    in_maps = []
    for c in range(NCORES):
        h_pad = np.zeros((NA, E), np.float32)
        x_pad = np.zeros((NA, 3), np.float32)
        kTc = np.zeros((SB, 3, K), np.float32)
        for j in range(SB):
            g = c * SB + j
            s, e = starts[g], starts[g + 1]
            nb = e - s
            h_pad[j * NS:j * NS + nb] = h[s:e]
            x_pad[j * NS:j * NS + nb] = x[s:e]
            kTc[j] = k[g].T
        in_maps.append({
            "hT": np.ascontiguousarray(h_pad.T),
            "xT": np.ascontiguousarray(x_pad.T),
            "kT": kTc,
            "kfil": kfil,
            "WpreT": WpreT,
            "W0T": W0T,
            "WresT": WresT,
        })

    if NS not in _cache:
        _cache[NS] = _build(NS)
    nc = _cache[NS]

    res = run_bass_kernel_spmd(nc, in_maps, list(range(NCORES)))

    hu = np.empty((N, E), np.float32)
    dot = np.empty((N, K), np.float32)
    for c in range(NCORES):
        r = res.results[c]
        for j in range(SB):
            g = c * SB + j
            s, e = starts[g], starts[g + 1]
            nb = e - s
            hu[s:e] = r["hu"][j * NS:j * NS + nb]
            dot[s:e] = r["dot"][j * NS:j * NS + nb]
    return hu, dot
